# revision 9
# baseline (speedup 1.0000x reference)
"""Trainium2 Bass kernel for nn_CapsuleSubLayer (capsule routing layer).

Full-input contract: kernel(x, weights) takes the FULL inputs
  x: (8, 8, 1024, 128) f32, weights: (8, 8, 128, 128) f32
and returns the full (8192, 1024) f32 output, distributing over 8
NeuronCores internally (data-parallel over the joint batch axis).

Algorithmic restructuring (validated numerically vs the reference):
  * Only x[-1] and weights[-1] matter: s/v use u_hat[:, -1] only, and
    C[-1]=softmax(B[-1]) uses row -1 of B only, whose update uses
    u_hat_mean[-1] only.
  * u_hat.mean(0) commutes with the linear map -> tiny matvec with the
    batch-mean of x[-1].
  * squash(c_j * u_hat) = scale(c_j, |u_hat|^2) * u_hat, so routing
    iterations only need per-row squared norms q and two small
    all-gathers (one per non-final routing round).
"""

import os
import sys
import numpy as np

for _p in ("/opt/trn_rl_repo",):
    if _p not in sys.path:
        sys.path.insert(0, _p)

P = 128          # partitions / in_dim / out_dim / seq block
NJ = 8           # num_out capsules
NT = 8           # row tiles per core (each 128 rows)
NCORES = 8
JB = 8192        # joint batch (bsz * seq)
ROWS = JB // NCORES   # rows per core = 1024
JE = NJ * P      # 1024 flattened (j, e)
EPS = 1e-8
INV_JB2 = 1.0 / (float(JB) * float(JB))

_CACHE = {}


def _build_nc():
    from concourse import bacc, tile, mybir
    stage = int(os.environ.get("KSTAGE", "4"))

    F32 = mybir.dt.float32

    nc = bacc.Bacc("TRN2", target_bir_lowering=False, debug=False,
                   num_devices=NCORES)

    xlt_d = nc.dram_tensor("xlt", [P, ROWS], F32, kind="ExternalInput")
    wmat_d = nc.dram_tensor("wmat", [P, JE], F32, kind="ExternalInput")
    out_d = nc.dram_tensor("out", [ROWS, JE], F32, kind="ExternalOutput")
    id8_d = nc.inline_tensor(np.eye(NJ, dtype=np.float32), name="id8c")

    with tile.TileContext(nc) as tc:
        with (
            tc.tile_pool(name="io", bufs=1) as io,
            tc.tile_pool(name="upool", bufs=1) as upool,
            tc.tile_pool(name="sq", bufs=2) as sqp,
            tc.tile_pool(name="small", bufs=1) as sm,
            tc.tile_pool(name="vout", bufs=3) as vp,
            tc.tile_pool(name="psum", bufs=2, space="PSUM") as pp,
            tc.tile_pool(name="pvmp", bufs=1, space="PSUM") as pvmp,
            tc.tile_pool(name="psmall", bufs=1, space="PSUM") as pps,
            tc.tile_pool(name="dram", bufs=1, space="DRAM") as dr,
        ):
            _body(nc, mybir, stage,
                  io, upool, sqp, sm, vp, pp, pvmp, pps, dr,
                  xlt_d, wmat_d, out_d, id8_d)

    nc.compile()
    return nc


def _body(nc, mybir, stage, io, upool, sqp, sm, vp, pp, pvmp, pps, dr,
          xlt_d, wmat_d, out_d, id8_d):
    F32 = mybir.dt.float32
    ALU = mybir.AluOpType
    ACTF = mybir.ActivationFunctionType
    AX = mybir.AxisListType

    # ---- DRAM bounce buffers for the collectives ----
    # rank block for ag1: row 0 = vm partial (flat je), row 1 = uhm partial
    ag1_in = dr.tile([2, JE], F32)
    ag1_out = dr.tile([NCORES, 2, NJ, P], F32, addr_space="Shared")
    ag2_in = dr.tile([1, JE], F32)
    ag2_out = dr.tile([NCORES, NJ, P], F32, addr_space="Shared")

    # ---- constants ----
    ones_row = sm.tile([1, P], F32)          # lhsT for bcast matmul
    nc.vector.memset(ones_row[:], 1.0)
    id8 = sm.tile([NJ, NJ], F32)             # for (8,1)->(1,8)
    nc.sync.dma_start(out=id8[:], in_=id8_d[:])
    zero_col = sm.tile([P, 1], F32)          # bias operands for ACT
    nc.vector.memset(zero_col[:], 0.0)
    eps_col = sm.tile([P, 1], F32)
    nc.vector.memset(eps_col[:], EPS)

    # ---- load inputs ----
    xlt = io.tile([P, ROWS], F32)            # (d, r)
    nc.sync.dma_start(out=xlt[:], in_=xlt_d[:])
    wmat = io.tile([P, JE], F32)             # (d, j*128+e)
    nc.sync.dma_start(out=wmat[:], in_=wmat_d[:])

    # ---- local batch-sum of x rows: m_col[d] = sum_r xlt[d, r] ----
    mscratch = sm.tile([P, ROWS], F32)
    m_col = sm.tile([P, 1], F32)
    nc.scalar.activation(mscratch[:], xlt[:], ACTF.Copy, accum_out=m_col[:])

    # ---- main matmul: U_t = xlt_t.T @ wmat, plus q ----
    u_tiles = []
    Q = sm.tile([P, NT * NJ], F32)           # q, columns t*8+j
    for t in range(NT):
        pu = pp.tile([P, JE], F32, tag="pu")
        for h in range(2):
            nc.tensor.matmul(
                pu[:, 512 * h:512 * (h + 1)],
                xlt[:, P * t:P * (t + 1)],
                wmat[:, 512 * h:512 * (h + 1)],
                start=True, stop=True)
        ut = upool.tile([P, JE], F32, tag=f"u{t}")
        nc.scalar.copy(ut[:], pu[:])
        sq = sqp.tile([P, JE], F32, tag="sq")
        nc.scalar.activation(sq[:], ut[:], ACTF.Square, bias=zero_col[:])
        nc.vector.tensor_reduce(
            Q[:, NJ * t:NJ * (t + 1)],
            sq[:].rearrange("p (j e) -> p j e", j=NJ),
            axis=AX.X, op=ALU.add)
        u_tiles.append(ut)

    def dump_u():
        for t in range(NT):
            nc.sync.dma_start(out=out_d[P * t:P * (t + 1), :],
                              in_=u_tiles[t][:])

    if stage == 1:
        dump_u()
        return

    # ---- UHM partial row: uhm[je] = sum_d m_col[d] * wmat[d, je] ----
    puhm = pvmp.tile([1, JE], F32, tag="pvm")
    for h in range(2):
        nc.tensor.matmul(puhm[:, 512 * h:512 * (h + 1)], m_col[:],
                         wmat[:, 512 * h:512 * (h + 1)],
                         start=True, stop=True)
    uhm_row = sm.tile([1, JE], F32)
    nc.scalar.copy(uhm_row[:], puhm[:])
    nc.sync.dma_start(out=ag1_in[1:2, :], in_=uhm_row[:])

    # ---- scale chain helper: S = c * t / ((1+t) sqrt(t+eps)),
    #      t = c^2 * q.  cmat/c2mat are (P, NJ) or None for c=1/8.
    def scale_chain(tag, cmat, c2mat):
        T = sm.tile([P, NT * NJ], F32, name=f"T_{tag}")
        if c2mat is None:
            nc.vector.tensor_scalar_mul(T[:], Q[:], 0.015625)
        else:
            nc.vector.tensor_mul(
                T[:].rearrange("p (t j) -> p t j", t=NT),
                Q[:].rearrange("p (t j) -> p t j", t=NT),
                c2mat[:, None, :].broadcast_to([P, NT, NJ]))
        sq1 = sm.tile([P, NT * NJ], F32, name=f"sq1_{tag}")
        nc.scalar.activation(sq1[:], T[:], ACTF.Sqrt, bias=eps_col[:])
        d1 = sm.tile([P, NT * NJ], F32, name=f"d1_{tag}")
        nc.scalar.activation(d1[:], T[:], ACTF.Copy, bias=1.0)
        w = sm.tile([P, NT * NJ], F32, name=f"w_{tag}")
        nc.vector.tensor_mul(w[:], sq1[:], d1[:])
        r = sm.tile([P, NT * NJ], F32, name=f"r_{tag}")
        nc.vector.reciprocal(r[:], w[:])
        s = sm.tile([P, NT * NJ], F32, name=f"s_{tag}")
        nc.vector.tensor_mul(s[:], T[:], r[:])
        sc = sm.tile([P, NT * NJ], F32, name=f"sc_{tag}")
        if cmat is None:
            nc.vector.tensor_scalar_mul(sc[:], s[:], 0.125)
        else:
            nc.vector.tensor_mul(
                sc[:].rearrange("p (t j) -> p t j", t=NT),
                s[:].rearrange("p (t j) -> p t j", t=NT),
                cmat[:, None, :].broadcast_to([P, NT, NJ]))
        return sc

    # ---- weighted batch-sum row: vm[je] = sum_r S[r, j] U[r, je],
    #      M=1 matmuls per (j, t); psum row (1, JE) on partition 0.
    def vm_partial(tag, S):
        pvm = pvmp.tile([1, JE], F32, tag="pvm")
        for j in range(NJ):
            for t in range(NT):
                nc.tensor.matmul(
                    pvm[:, P * j:P * (j + 1)],
                    S[:, NJ * t + j:NJ * t + j + 1],
                    u_tiles[t][:, P * j:P * (j + 1)],
                    start=(t == 0), stop=(t == NT - 1))
        vm = sm.tile([1, JE], F32, name=f"vm_{tag}")
        nc.scalar.copy(vm[:], pvm[:])
        return vm

    # ---- softmax over j of a (1, NJ) logits row -> replicated (P, NJ) ----
    def softmax_rep(tag, brow):
        es = sm.tile([1, NJ + 1], F32, name=f"es_{tag}")
        nc.scalar.activation(es[:, 0:NJ], brow[:], ACTF.Exp,
                             bias=zero_col[0:1, :])
        nc.vector.tensor_reduce(es[:, NJ:NJ + 1], es[:, 0:NJ],
                                axis=AX.X, op=ALU.add)
        ep = pps.tile([P, NJ + 1], F32, tag="ep")
        nc.tensor.matmul(ep[:], ones_row[:], es[:], start=True, stop=True)
        rcp = sm.tile([P, 1], F32, name=f"rcp_{tag}")
        nc.vector.reciprocal(rcp[:], ep[:, NJ:NJ + 1])
        cmat = sm.tile([P, NJ], F32, name=f"cmat_{tag}")
        nc.vector.tensor_mul(cmat[:], ep[:, 0:NJ],
                             rcp[:].broadcast_to([P, NJ]))
        c2mat = sm.tile([P, NJ], F32, name=f"c2mat_{tag}")
        nc.vector.tensor_mul(c2mat[:], cmat[:], cmat[:])
        return cmat, c2mat

    # ---- shared tail after each all-gather: global-sum the vm payload,
    #      dot with UHMG, turn into a (1, NJ) logits row.
    def upd_row(tag, src_ap, uhmg):
        raw = sm.tile([NJ, NCORES * P], F32, name=f"raw_{tag}")
        nc.sync.dma_start(
            out=raw[:].rearrange("j (r e) -> j r e", r=NCORES),
            in_=src_ap)
        vmg = sm.tile([NJ, P], F32, name=f"vmg_{tag}")
        nc.vector.tensor_reduce(
            vmg[:], raw[:].rearrange("j (r e) -> j e r", r=NCORES),
            axis=AX.X, op=ALU.add)
        ttr = sm.tile([NJ, P], F32, name=f"ttr_{tag}")
        upd = sm.tile([NJ, 1], F32, name=f"upd_{tag}")
        nc.vector.tensor_mul(ttr[:], uhmg[:], vmg[:])
        nc.vector.tensor_reduce(upd[:], ttr[:], axis=AX.X, op=ALU.add)
        prow = pps.tile([1, NJ], F32, tag="prow")
        nc.tensor.matmul(prow[:], upd[:], id8[:], start=True, stop=True)
        urow = sm.tile([1, NJ], F32, name=f"urow_{tag}")
        nc.vector.tensor_scalar_mul(urow[:], prow[:], INV_JB2)
        return urow

    # ================= iteration 0 (c = 1/8 exactly) ============
    S0 = scale_chain("it0", None, None)
    vm0 = vm_partial("it0", S0)
    if stage == 2:
        dump_u()
        nc.sync.dma_start(out=out_d[0:1, :], in_=vm0[:])
        return
    nc.sync.dma_start(out=ag1_in[0:1, :], in_=vm0[:])

    nc.gpsimd.collective_compute(
        "AllGather", ALU.bypass,
        replica_groups=[list(range(NCORES))],
        ins=[ag1_in.opt()], outs=[ag1_out.opt()])

    # UHMG global (NJ, P)
    uhmg_raw = sm.tile([NJ, NCORES * P], F32)
    nc.sync.dma_start(
        out=uhmg_raw[:].rearrange("j (r e) -> j r e", r=NCORES),
        in_=ag1_out[:, 1, :, :].rearrange("r j e -> j r e"))
    UHMG = sm.tile([NJ, P], F32)
    nc.vector.tensor_reduce(
        UHMG[:], uhmg_raw[:].rearrange("j (r e) -> j e r", r=NCORES),
        axis=AX.X, op=ALU.add)

    b1row = upd_row("it0",
                    ag1_out[:, 0, :, :].rearrange("r j e -> j r e"),
                    UHMG)
    c1, c1sq = softmax_rep("it1", b1row)
    if stage == 3:
        dump_u()
        nc.sync.dma_start(out=out_d[0:P, 0:NJ], in_=c1[:])
        return

    # ================= iteration 1 ==============================
    S1 = scale_chain("it1", c1, c1sq)
    vm1 = vm_partial("it1", S1)
    nc.sync.dma_start(out=ag2_in[0:1, :], in_=vm1[:])

    nc.gpsimd.collective_compute(
        "AllGather", ALU.bypass,
        replica_groups=[list(range(NCORES))],
        ins=[ag2_in.opt()], outs=[ag2_out.opt()])

    u1row = upd_row("it1",
                    ag2_out[:, :, :].rearrange("r j e -> j r e"),
                    UHMG)
    b2row = sm.tile([1, NJ], F32)
    nc.vector.tensor_add(b2row[:], u1row[:], b1row[:])
    c2, c2sq = softmax_rep("it2", b2row)

    # ================= iteration 2: final output ================
    S2 = scale_chain("it2", c2, c2sq)
    for t in range(NT):
        vt = vp.tile([P, JE], F32, tag="vt")
        nc.vector.tensor_mul(
            vt[:].rearrange("p (j e) -> p j e", j=NJ),
            u_tiles[t][:].rearrange("p (j e) -> p j e", j=NJ),
            S2[:, NJ * t:NJ * (t + 1)][:, :, None]
                .broadcast_to([P, NJ, P]))
        nc.sync.dma_start(out=out_d[P * t:P * (t + 1), :], in_=vt[:])


def _get_nc():
    if "nc" not in _CACHE:
        _CACHE["nc"] = _build_nc()
    return _CACHE["nc"]


def _shard_inputs(x, weights):
    x7 = np.asarray(x)[-1]           # (8 b, 1024 s, 128 d)
    w7 = np.asarray(weights)[-1]     # (8 j, 128 d, 128 e)
    wmat = np.ascontiguousarray(
        w7.transpose(1, 0, 2).reshape(P, JE)).astype(np.float32, copy=False)
    in_maps = []
    for k in range(NCORES):
        sl = x7[:, P * k:P * (k + 1), :]          # (b, s_loc, d)
        xlt = np.ascontiguousarray(
            sl.transpose(2, 1, 0).reshape(P, ROWS)).astype(
                np.float32, copy=False)           # (d, r) r = s*8+b
        in_maps.append({"xlt": xlt, "wmat": wmat})
    return in_maps


def _run(x, weights, trace=False, trace_kwargs=None, tmpdir=None):
    from concourse import bass_utils
    nc = _get_nc()
    in_maps = _shard_inputs(x, weights)
    res = bass_utils.run_bass_kernel_spmd(
        nc, in_maps, list(range(NCORES)), trace=trace,
        tmpdir=tmpdir, **(trace_kwargs or {}))
    _CACHE["last_results"] = res
    out = np.empty((JB, JE), dtype=np.float32)
    for k in range(NCORES):
        out[ROWS * k:ROWS * (k + 1), :] = res.results[k]["out"]
    return out


def kernel(x, weights):
    return _run(x, weights, trace=False)
